# revision 25
# baseline (speedup 1.0000x reference)
"""Biaffine label attention kernel for 8 Trainium2 NeuronCores.

Math (per batch b, label l):
    out[b,l,i,o] = sum_d head[b,i,d] * U[l,d] * dep[b,o,d]
                 + sum_d head[b,i,d] * Wh[l,d]
                 + sum_d dep[b,o,d]  * Wd[l,d]
                 + bias[l]

Device-side rewrite: with M[d,o] = U[l,d]*dep[b,o,d] + Wh[l,d] (one fused
DVE tensor_scalar per 128-row chunk) the first two terms become a single
K=768 contraction.  The kernel computes the TRANSPOSED plane

    outT[o,i] = sum_d M[d,o] * headT[d,i]  + augT[o,l]

so the leftover broadcast term augT[o,l] = t2_d[l,o] + bias[l] varies
along PSUM *partitions* and is added for free by the per-partition `bias`
operand of the ScalarE PSUM->SBUF copy.  The host returns a zero-copy
transposed view to restore [i,o] order.

Sharding: labels split 8-ways (8 labels per core); every core sees all 4
batches and writes its own [4, 8, 512, 512] fp32 output block.

Toolchain quirks handled below:
  - float32r ("rounded" fp32, ~tf32 precision) runs the PE at full rate
    but every tensor feeding a matmul must be produced as float32r.
  - walrus caps sync waits at 1 per ISA instruction: `absorb()` dummies
    pre-pull DMA completions into each consuming engine's vector clock,
    and `_split_waits` hoists any remaining excess waits onto standalone
    EventSemaphore instructions.
  - f32r matmuls need an even moving free dim (N>=2).
"""

import numpy as np

B, S, D, L = 4, 512, 768, 64
NCORES = 8
LC = L // NCORES      # labels per core
P = 128               # partitions
DC = D // P           # contraction chunks of 128

MM_DTYPE = "f32r"

_CACHE = {}


def _build_nc():
    import concourse.bass as bass
    import concourse.mybir as mybir
    import concourse.tile as tile

    f32 = mybir.dt.float32
    mmdt = {
        "f32r": mybir.dt.float32r,
        "bf16": mybir.dt.bfloat16,
        "f32": f32,
    }[MM_DTYPE]
    Ident = mybir.ActivationFunctionType.Identity

    nc = bass.Bass(target_bir_lowering=False)

    head_t = nc.dram_tensor("head_t", [B, P, DC, S], mmdt, kind="ExternalInput")
    dep_t = nc.dram_tensor("dep_t", [B, P, DC, S], mmdt, kind="ExternalInput")
    # packed consts: dve_c = [u | wh] ; pe_c = [wd | bias,ones on row 0]
    dve_c_t = nc.dram_tensor(
        "dve_c_t", [P, 2, DC, LC], f32, kind="ExternalInput"
    )
    pe_c_t = nc.dram_tensor(
        "pe_c_t", [P, DC * LC + LC + P], mmdt, kind="ExternalInput"
    )
    # out is the TRANSPOSED plane: outT[b, l, o, i]
    out_t = nc.dram_tensor("out", [B, LC, S, S], f32, kind="ExternalOutput")

    with (
        tile.TileContext(nc) as tc,
        tc.tile_pool(name="const", bufs=1) as constp,
        tc.tile_pool(name="io", bufs=2) as iop,
        tc.tile_pool(name="m", bufs=3) as mp,
        tc.tile_pool(name="o", bufs=3) as op,
        tc.tile_pool(name="ps", bufs=5, space="PSUM") as psp,
        tc.tile_pool(name="psa", bufs=2, space="PSUM") as psap,
        tc.tile_pool(name="pssc", bufs=1, space="PSUM") as pssc,
    ):
        sc_tile = pssc.tile([1, 64], f32, tag="sc")
        scs_tile = constp.tile([1, 64], f32, tag="scs")
        absorb_n = [0]

        def absorb(tile_ap, eng="pe"):
            """Tiny op reading `tile_ap` so the consuming engine's vector
            clock covers the producer; real instructions downstream then
            need at most the single sync wait walrus allows."""
            j = absorb_n[0]
            absorb_n[0] += 1
            if eng == "pe":
                jj = (j % 32) * 2
                nc.tensor.matmul(
                    sc_tile[:, jj : jj + 2],
                    tile_ap[0:1, 0:1],
                    tile_ap[0:1, 0:2],
                    start=True,
                    stop=True,
                )
            elif eng == "dve":
                nc.vector.tensor_copy(
                    scs_tile[:, j % 64 : j % 64 + 1], tile_ap[0:1, 0:1]
                )
            elif eng == "act":
                nc.scalar.activation(
                    scs_tile[:, j % 64 : j % 64 + 1], tile_ap[0:1, 0:1], Ident
                )

        pe_c = constp.tile([P, DC * LC + LC + P], mmdt)
        nc.sync.dma_start(pe_c[:], pe_c_t[:])
        dve_c = constp.tile([P, 2, DC, LC], f32)
        nc.sync.dma_start(dve_c[:], dve_c_t[:])
        u_sb = dve_c[:, 0]
        wh_sb = dve_c[:, 1]
        wd_sb = pe_c[:, : DC * LC].rearrange("p (c l) -> p c l", c=DC)
        bias_sb = pe_c[0:1, DC * LC : DC * LC + LC]
        ones_sb = pe_c[0:1, DC * LC + LC :]
        absorb(pe_c[:, 0:2])
        absorb(dve_c[:, 0, 0, :], "dve")

        for b in range(B):
            dT = iop.tile([P, DC, S], mmdt, tag="dT")
            nc.sync.dma_start(dT[:], dep_t[b])
            hT = iop.tile([P, DC, S], mmdt, tag="hT")
            nc.sync.dma_start(hT[:], head_t[b])
            absorb(dT[:, 0, :])
            absorb(dT[:, 0, :], "dve")

            # augT[o, l] = t2_d[l, o] + bias[l], per o-block
            augT = iop.tile([P, 4, LC], f32, tag="augT")
            for ob in range(4):
                ps_a = psap.tile([P, LC], f32, tag="psa")
                for c in range(DC):
                    nc.tensor.matmul(
                        ps_a[:],
                        dT[:, c, ob * P : (ob + 1) * P],
                        wd_sb[:, c, :],
                        start=(c == 0),
                        stop=False,
                    )
                # += 1[o] * bias[l]
                nc.tensor.matmul(
                    ps_a[:], ones_sb, bias_sb, start=False, stop=True
                )
                nc.scalar.activation(augT[:, ob, :], ps_a[:], Ident)
            absorb(hT[:, 0, :])

            for l in range(LC):
                # M[d, o] = U[l,d] * depT[d,o] + Wh[l,d]
                m_t = mp.tile([P, DC, S], mmdt, tag="m")
                for c in range(DC):
                    nc.vector.tensor_scalar(
                        m_t[:, c, :],
                        dT[:, c, :],
                        u_sb[:, c, l : l + 1],
                        wh_sb[:, c, l : l + 1],
                        mybir.AluOpType.mult,
                        mybir.AluOpType.add,
                    )
                o_t = op.tile([P, 4, S], f32, tag="o")
                for ob in range(4):
                    ps = psp.tile([P, S], f32, tag="ps")
                    for c in range(DC):
                        nc.tensor.matmul(
                            ps[:],
                            m_t[:, c, ob * P : (ob + 1) * P],
                            hT[:, c, :],
                            start=(c == 0),
                            stop=(c == DC - 1),
                        )
                    # copy + broadcast-add of augT via per-partition bias
                    nc.scalar.activation(
                        o_t[:, ob, :], ps[:], Ident, bias=augT[:, ob, l : l + 1]
                    )
                    if b == B - 1 and l == LC - 1:
                        nc.sync.dma_start(
                            out_t[b, l].rearrange("(ob p) i -> p ob i", p=P)[
                                :, ob, :
                            ],
                            o_t[:, ob, :],
                        )
                if not (b == B - 1 and l == LC - 1):
                    nc.sync.dma_start(
                        out_t[b, l].rearrange("(ob p) i -> p ob i", p=P), o_t[:]
                    )
    return nc


def _split_waits(nc):
    """Walrus in this toolchain allows a single sync wait per ISA
    instruction.  Hoist excess waits onto standalone EventSemaphore
    instructions on the same engine, which execute on the engine's
    sequencer in program order just before the instruction."""
    import concourse.mybir as mybir

    n = [0]
    for fn in nc.m.functions:
        for bb in fn.blocks:
            insts = bb.instructions
            out = []
            changed = False
            for inst in insts:
                si = inst.sync_info
                waits = list(si.on_wait) if si and si.on_wait else []
                if len(waits) > 1:
                    for w in waits[:-1]:
                        ev = mybir.InstEventSemaphore(
                            name=f"wsplit_{n[0]}", ins=[], outs=[]
                        )
                        n[0] += 1
                        ev.engine = inst.engine
                        ev.sync_info = mybir.SyncInfo(on_wait=[w], on_update=[])
                        out.append(ev)
                    inst.sync_info = mybir.SyncInfo(
                        on_wait=waits[-1:], on_update=list(si.on_update or [])
                    )
                    changed = True
                out.append(inst)
            if changed:
                bb.instructions = out
    return nc


def _get_nc():
    if "nc" not in _CACHE:
        _CACHE["nc"] = _split_waits(_build_nc())
    return _CACHE["nc"]


def _prep_dxs(x):
    # [B, S, D] -> [B, P, DC, S] with x_t[b, p, c, s] = x[b, s, c*P + p]
    xt = np.transpose(np.asarray(x, np.float32), (0, 2, 1))  # [B, D, S]
    xt = xt.reshape(B, DC, P, S).transpose(0, 2, 1, 3)
    return np.ascontiguousarray(xt)


def _pack_pe_consts(wd, bias):
    out = np.zeros((P, DC * LC + LC + P), np.float32)
    out[:, : DC * LC] = _prep_w(wd).reshape(P, DC * LC)
    out[0, DC * LC : DC * LC + LC] = bias.astype(np.float32)
    out[0, DC * LC + LC :] = 1.0
    return np.ascontiguousarray(out)


def _prep_w(w):
    # [LC, D] -> [P, DC, LC] with w_t[p, c, l] = w[l, c*P + p]
    wt = np.asarray(w, np.float32).T.reshape(DC, P, LC).transpose(1, 0, 2)
    return np.ascontiguousarray(wt)


LAST_RESULT = None


def kernel(head, dep, label_U_diag, label_W, label_b, **_unused):
    import os

    from concourse.bass_utils import run_bass_kernel_spmd

    head = np.asarray(head, np.float32)
    dep = np.asarray(dep, np.float32)
    label_U_diag = np.asarray(label_U_diag, np.float32)
    label_W = np.asarray(label_W, np.float32)
    label_b = np.asarray(label_b, np.float32)

    head_np = _prep_dxs(head)
    dep_np = _prep_dxs(dep)

    in_maps = []
    for c in range(NCORES):
        lo, hi = c * LC, (c + 1) * LC
        in_maps.append(
            {
                "head_t": head_np,
                "dep_t": dep_np,
                "dve_c_t": np.ascontiguousarray(
                    np.stack(
                        [
                            _prep_w(label_U_diag[lo:hi]),
                            _prep_w(label_W[lo:hi, :D]),
                        ],
                        axis=1,
                    )
                ),
                "pe_c_t": _pack_pe_consts(
                    label_W[lo:hi, D:], label_b[lo:hi]
                ),
            }
        )

    nc = _get_nc()
    trace = bool(os.environ.get("BIAFFINE_TRACE"))

    def run_once():
        try:
            return run_bass_kernel_spmd(
                nc, in_maps, core_ids=list(range(NCORES)), trace=trace
            )
        except (ImportError, ModuleNotFoundError):
            # NTFF profiling hook unavailable in this environment
            return run_bass_kernel_spmd(nc, in_maps, core_ids=list(range(NCORES)))

    def spot_check(out):
        # Re-derive a few output elements in float64 on the host, one per
        # core, to catch transient transport/execution corruption.
        h64 = head.astype(np.float64)
        d64 = dep.astype(np.float64)
        U64 = label_U_diag.astype(np.float64)
        W64 = label_W.astype(np.float64)
        b64 = label_b.astype(np.float64)
        for c in range(NCORES):
            l = c * LC + (c * 3) % LC
            for b, i, o in ((c % B, 17 + c, 200), ((c + 1) % B, 400, 31 * c + 5)):
                v = (
                    np.dot(h64[b, i] * U64[l], d64[b, o])
                    + np.dot(h64[b, i], W64[l, :D])
                    + np.dot(d64[b, o], W64[l, D:])
                    + b64[l]
                )
                got = float(out[b, l, i, o])
                if abs(got - v) > 0.05 + 0.01 * abs(v):
                    return False
        return True

    global LAST_RESULT
    out = None
    for attempt in range(3):
        try:
            res = run_once()
        except Exception:
            if attempt == 2:
                raise
            continue
        LAST_RESULT = res
        outT = np.concatenate([r["out"] for r in res.results], axis=1)
        # device wrote transposed planes [o, i]; restore [i, o] as a view
        out = outT.transpose(0, 1, 3, 2)
        if spot_check(out):
            return out
    return out


# revision 27
# speedup vs baseline: 1.0008x; 1.0008x over previous
"""Biaffine label attention kernel for 8 Trainium2 NeuronCores.

Math (per batch b, label l):
    out[b,l,i,o] = sum_d head[b,i,d] * U[l,d] * dep[b,o,d]
                 + sum_d head[b,i,d] * Wh[l,d]
                 + sum_d dep[b,o,d]  * Wd[l,d]
                 + bias[l]

Device-side rewrite: with M[d,o] = U[l,d]*dep[b,o,d] + Wh[l,d] (one fused
DVE tensor_scalar per 128-row chunk) the first two terms become a single
K=768 contraction.  The kernel computes the TRANSPOSED plane

    outT[o,i] = sum_d M[d,o] * headT[d,i]  + augT[o,l]

so the leftover broadcast term augT[o,l] = t2_d[l,o] + bias[l] varies
along PSUM *partitions* and is added for free by the per-partition `bias`
operand of the ScalarE PSUM->SBUF copy.  The host returns a zero-copy
transposed view to restore [i,o] order.

Sharding: labels split 8-ways (8 labels per core); every core sees all 4
batches and writes its own [4, 8, 512, 512] fp32 output block.

Toolchain quirks handled below:
  - float32r ("rounded" fp32, ~tf32 precision) runs the PE at full rate
    but every tensor feeding a matmul must be produced as float32r.
  - walrus caps sync waits at 1 per ISA instruction: `absorb()` dummies
    pre-pull DMA completions into each consuming engine's vector clock,
    and `_split_waits` hoists any remaining excess waits onto standalone
    EventSemaphore instructions.
  - f32r matmuls need an even moving free dim (N>=2).
"""

import numpy as np

B, S, D, L = 4, 512, 768, 64
NCORES = 8
LC = L // NCORES      # labels per core
P = 128               # partitions
DC = D // P           # contraction chunks of 128

MM_DTYPE = "f32r"

_CACHE = {}


def _build_nc():
    import concourse.bass as bass
    import concourse.mybir as mybir
    import concourse.tile as tile

    f32 = mybir.dt.float32
    mmdt = {
        "f32r": mybir.dt.float32r,
        "bf16": mybir.dt.bfloat16,
        "f32": f32,
    }[MM_DTYPE]
    Ident = mybir.ActivationFunctionType.Identity

    nc = bass.Bass(target_bir_lowering=False)

    head_t = nc.dram_tensor("head_t", [B, P, DC, S], mmdt, kind="ExternalInput")
    dep_t = nc.dram_tensor("dep_t", [B, P, DC, S], mmdt, kind="ExternalInput")
    # packed consts: dve_c = [u | wh] ; pe_c = [wd | bias,ones on row 0]
    dve_c_t = nc.dram_tensor(
        "dve_c_t", [P, 2, DC, LC], f32, kind="ExternalInput"
    )
    pe_c_t = nc.dram_tensor(
        "pe_c_t", [P, DC * LC + LC + P], mmdt, kind="ExternalInput"
    )
    # out is the TRANSPOSED plane: outT[b, l, o, i]
    out_t = nc.dram_tensor("out", [B, LC, S, S], f32, kind="ExternalOutput")

    with (
        tile.TileContext(nc) as tc,
        tc.tile_pool(name="const", bufs=1) as constp,
        tc.tile_pool(name="io", bufs=2) as iop,
        tc.tile_pool(name="m", bufs=3) as mp,
        tc.tile_pool(name="o", bufs=3) as op,
        tc.tile_pool(name="ps", bufs=5, space="PSUM") as psp,
        tc.tile_pool(name="psa", bufs=2, space="PSUM") as psap,
        tc.tile_pool(name="pssc", bufs=1, space="PSUM") as pssc,
    ):
        sc_tile = pssc.tile([1, 64], f32, tag="sc")
        scs_tile = constp.tile([1, 64], f32, tag="scs")
        absorb_n = [0]

        def absorb(tile_ap, eng="pe"):
            """Tiny op reading `tile_ap` so the consuming engine's vector
            clock covers the producer; real instructions downstream then
            need at most the single sync wait walrus allows."""
            j = absorb_n[0]
            absorb_n[0] += 1
            if eng == "pe":
                jj = (j % 32) * 2
                nc.tensor.matmul(
                    sc_tile[:, jj : jj + 2],
                    tile_ap[0:1, 0:1],
                    tile_ap[0:1, 0:2],
                    start=True,
                    stop=True,
                )
            elif eng == "dve":
                nc.vector.tensor_copy(
                    scs_tile[:, j % 64 : j % 64 + 1], tile_ap[0:1, 0:1]
                )
            elif eng == "act":
                nc.scalar.activation(
                    scs_tile[:, j % 64 : j % 64 + 1], tile_ap[0:1, 0:1], Ident
                )

        pe_c = constp.tile([P, DC * LC + LC + P], mmdt)
        nc.sync.dma_start(pe_c[:], pe_c_t[:])
        dve_c = constp.tile([P, 2, DC, LC], f32)
        nc.sync.dma_start(dve_c[:], dve_c_t[:])
        u_sb = dve_c[:, 0]
        wh_sb = dve_c[:, 1]
        wd_sb = pe_c[:, : DC * LC].rearrange("p (c l) -> p c l", c=DC)
        bias_sb = pe_c[0:1, DC * LC : DC * LC + LC]
        ones_sb = pe_c[0:1, DC * LC + LC :]
        absorb(pe_c[:, 0:2])
        absorb(dve_c[:, 0, 0, :], "dve")

        for b in range(B):
            dT = iop.tile([P, DC, S], mmdt, tag="dT")
            nc.sync.dma_start(dT[:], dep_t[b])
            hT = iop.tile([P, DC, S], mmdt, tag="hT")
            nc.sync.dma_start(hT[:], head_t[b])
            absorb(dT[:, 0, :])
            absorb(dT[:, 0, :], "dve")

            # augT[o, l] = t2_d[l, o] + bias[l], per o-block
            augT = iop.tile([P, 4, LC], f32, tag="augT")
            for ob in range(4):
                ps_a = psap.tile([P, LC], f32, tag="psa")
                for c in range(DC):
                    nc.tensor.matmul(
                        ps_a[:],
                        dT[:, c, ob * P : (ob + 1) * P],
                        wd_sb[:, c, :],
                        start=(c == 0),
                        stop=False,
                    )
                # += 1[o] * bias[l]
                nc.tensor.matmul(
                    ps_a[:], ones_sb, bias_sb, start=False, stop=True
                )
                nc.scalar.activation(augT[:, ob, :], ps_a[:], Ident)
            absorb(hT[:, 0, :])

            for l in range(LC):
                # M[d, o] = U[l,d] * depT[d,o] + Wh[l,d]
                m_t = mp.tile([P, DC, S], mmdt, tag="m")
                for c in range(DC):
                    nc.vector.tensor_scalar(
                        m_t[:, c, :],
                        dT[:, c, :],
                        u_sb[:, c, l : l + 1],
                        wh_sb[:, c, l : l + 1],
                        mybir.AluOpType.mult,
                        mybir.AluOpType.add,
                    )
                o_t = op.tile([P, 4, S], f32, tag="o")
                for ob in range(4):
                    ps = psp.tile([P, S], f32, tag="ps")
                    for c in range(DC):
                        nc.tensor.matmul(
                            ps[:],
                            m_t[:, c, ob * P : (ob + 1) * P],
                            hT[:, c, :],
                            start=(c == 0),
                            stop=(c == DC - 1),
                        )
                    # copy + broadcast-add of augT via per-partition bias
                    nc.scalar.activation(
                        o_t[:, ob, :], ps[:], Ident, bias=augT[:, ob, l : l + 1]
                    )
                    if b == B - 1 and l >= LC - 2:
                        nc.sync.dma_start(
                            out_t[b, l].rearrange("(ob p) i -> p ob i", p=P)[
                                :, ob, :
                            ],
                            o_t[:, ob, :],
                        )
                if not (b == B - 1 and l >= LC - 2):
                    nc.sync.dma_start(
                        out_t[b, l].rearrange("(ob p) i -> p ob i", p=P), o_t[:]
                    )
    return nc


def _split_waits(nc):
    """Walrus in this toolchain allows a single sync wait per ISA
    instruction.  Hoist excess waits onto standalone EventSemaphore
    instructions on the same engine, which execute on the engine's
    sequencer in program order just before the instruction."""
    import concourse.mybir as mybir

    n = [0]
    for fn in nc.m.functions:
        for bb in fn.blocks:
            insts = bb.instructions
            out = []
            changed = False
            for inst in insts:
                si = inst.sync_info
                waits = list(si.on_wait) if si and si.on_wait else []
                if len(waits) > 1:
                    for w in waits[:-1]:
                        ev = mybir.InstEventSemaphore(
                            name=f"wsplit_{n[0]}", ins=[], outs=[]
                        )
                        n[0] += 1
                        ev.engine = inst.engine
                        ev.sync_info = mybir.SyncInfo(on_wait=[w], on_update=[])
                        out.append(ev)
                    inst.sync_info = mybir.SyncInfo(
                        on_wait=waits[-1:], on_update=list(si.on_update or [])
                    )
                    changed = True
                out.append(inst)
            if changed:
                bb.instructions = out
    return nc


def _get_nc():
    if "nc" not in _CACHE:
        _CACHE["nc"] = _split_waits(_build_nc())
    return _CACHE["nc"]


def _prep_dxs(x):
    # [B, S, D] -> [B, P, DC, S] with x_t[b, p, c, s] = x[b, s, c*P + p]
    xt = np.transpose(np.asarray(x, np.float32), (0, 2, 1))  # [B, D, S]
    xt = xt.reshape(B, DC, P, S).transpose(0, 2, 1, 3)
    return np.ascontiguousarray(xt)


def _pack_pe_consts(wd, bias):
    out = np.zeros((P, DC * LC + LC + P), np.float32)
    out[:, : DC * LC] = _prep_w(wd).reshape(P, DC * LC)
    out[0, DC * LC : DC * LC + LC] = bias.astype(np.float32)
    out[0, DC * LC + LC :] = 1.0
    return np.ascontiguousarray(out)


def _prep_w(w):
    # [LC, D] -> [P, DC, LC] with w_t[p, c, l] = w[l, c*P + p]
    wt = np.asarray(w, np.float32).T.reshape(DC, P, LC).transpose(1, 0, 2)
    return np.ascontiguousarray(wt)


LAST_RESULT = None


def kernel(head, dep, label_U_diag, label_W, label_b, **_unused):
    import os

    from concourse.bass_utils import run_bass_kernel_spmd

    head = np.asarray(head, np.float32)
    dep = np.asarray(dep, np.float32)
    label_U_diag = np.asarray(label_U_diag, np.float32)
    label_W = np.asarray(label_W, np.float32)
    label_b = np.asarray(label_b, np.float32)

    head_np = _prep_dxs(head)
    dep_np = _prep_dxs(dep)

    in_maps = []
    for c in range(NCORES):
        lo, hi = c * LC, (c + 1) * LC
        in_maps.append(
            {
                "head_t": head_np,
                "dep_t": dep_np,
                "dve_c_t": np.ascontiguousarray(
                    np.stack(
                        [
                            _prep_w(label_U_diag[lo:hi]),
                            _prep_w(label_W[lo:hi, :D]),
                        ],
                        axis=1,
                    )
                ),
                "pe_c_t": _pack_pe_consts(
                    label_W[lo:hi, D:], label_b[lo:hi]
                ),
            }
        )

    nc = _get_nc()
    trace = bool(os.environ.get("BIAFFINE_TRACE"))

    def run_once():
        try:
            return run_bass_kernel_spmd(
                nc, in_maps, core_ids=list(range(NCORES)), trace=trace
            )
        except (ImportError, ModuleNotFoundError):
            # NTFF profiling hook unavailable in this environment
            return run_bass_kernel_spmd(nc, in_maps, core_ids=list(range(NCORES)))

    def spot_check(out):
        # Re-derive a few output elements in float64 on the host, one per
        # core, to catch transient transport/execution corruption.
        h64 = head.astype(np.float64)
        d64 = dep.astype(np.float64)
        U64 = label_U_diag.astype(np.float64)
        W64 = label_W.astype(np.float64)
        b64 = label_b.astype(np.float64)
        for c in range(NCORES):
            l = c * LC + (c * 3) % LC
            for b, i, o in ((c % B, 17 + c, 200), ((c + 1) % B, 400, 31 * c + 5)):
                v = (
                    np.dot(h64[b, i] * U64[l], d64[b, o])
                    + np.dot(h64[b, i], W64[l, :D])
                    + np.dot(d64[b, o], W64[l, D:])
                    + b64[l]
                )
                got = float(out[b, l, i, o])
                if abs(got - v) > 0.05 + 0.01 * abs(v):
                    return False
        return True

    global LAST_RESULT
    out = None
    for attempt in range(3):
        try:
            res = run_once()
        except Exception:
            if attempt == 2:
                raise
            continue
        LAST_RESULT = res
        outT = np.concatenate([r["out"] for r in res.results], axis=1)
        # device wrote transposed planes [o, i]; restore [i, o] as a view
        out = outT.transpose(0, 1, 3, 2)
        if spot_check(out):
            return out
    return out
